# revision 21
# baseline (speedup 1.0000x reference)
"""Trainium2 Bass kernel for nn_NumAttention (sparse_attention).

Reference computation (per batch b, head i):
    k     = blockmix(x_cat, softmax(W_K)[i])            # [P, DH]
    xq    = blockmix(x_cat, softmax(W_Q)[i])            # [P, DH]
    q     = xq @ softmax(W_pred)[i]                     # [P, DH]
    v     = x_num @ softmax(W_V)[i]                     # [P]
    z[qp] = sum_{p<=qp} v[p] * (k[p] . q[qp])           # causal, no softmax

The attention is softmax-free with scalar values, hence linear:
z[qp] = xq[qp] . S[qp] with S = cumsum_p(v[p] * ktilde[p,:]),
ktilde = k @ pp^T folded into one mix weight.  O(P^2) scores never exist.

Design notes:
  * Features are reordered h-major (f' = h*NV + v) so the q-side weight
    pq (x) I is block-diagonal: each 128-feature K-tile feeds exactly 64
    q columns -> four N=64 matmuls per chunk instead of a dense N=256
    stream.  Mix = 20480 PE columns instead of 32768.
  * ktilde/q columns are ordered (chunk, head, t, glo): the final
    z[p,(c,i)] = sum_64(q * S) is one tensor_tensor + one tensor_reduce
    per chunk-pair, writing z columns directly (no fold pass).  The q
    matmuls scatter into a strided PSUM view to interleave t=kc.
  * 16 chunks run as four uneven quarters (6/4/4/2) with a sequential
    inter-quarter carry: pass-2 of quarter Q overlaps the mix of Q+1, and
    the tiny tail quarter skips the whole prefix pipeline (its TexL IS the
    carry; the odd chunk's total arrives as a ones-matrix broadcast matmul
    straight into PSUM).
  * Mix psum is allocated per chunk-PAIR (one k bank, one q bank) so the
    post-matmul DVE/ACT traffic is one vk-multiply + one q-copy per pair.
  * All weights+constants ship as ONE [128, 1296] DMA (2.6KB/partition
    descriptors); x ships as 8 pair-slices on the other HWDGE ring.

Sharding: 8 cores = 4 batches x 2 head-groups (4 heads each).  Host ships
x_cat[b] transposed h-major bf16 and host-computed v (x_num @ pv, 8 MFLOP).
A burst of skinny dummy matmuls warms the PE HAM clock gate during the DMA
head.
"""

import numpy as np
import ml_dtypes

import concourse.bacc as bacc
import concourse.mybir as mybir
import concourse.tile as tile
from concourse.bass_utils import run_bass_kernel_spmd

B, P, DC, DN, H, DH = 4, 2048, 512, 64, 8, 64
NV = DC // DH     # 8 variables
CH = 128          # positions per chunk
NCH = P // CH     # 16 chunks
HPC = 4           # heads per core
NT = 4            # h-groups of 16 (= KC)
GL = 16           # h's per group
FH = HPC * DH     # 256 = ktilde/q width per core, cols ordered (i, t, glo)
FH2 = 2 * FH      # 512 = chunk-pair width
KC = DC // CH     # 4 feature K-tiles
QW = HPC * GL     # 64 q-columns per K-tile
NQ = 4            # quarters (uneven: big early, tiny last for a short tail)
CPQS = (6, 4, 4, 2)            # chunks per quarter
NPQS = (3, 2, 2, 1)            # pairs per quarter
CB = (0, 6, 10, 14)            # chunk base per quarter
PB = (0, 3, 5, 7)              # pair base per quarter
OH_OFF = (0, 36, 52, 68)       # oneh col offset per quarter (cpq^2 widths)
OHW = 72
SEL_W = 3 * CH                 # sel block in cs (up to 3 pairs)
ST_OFF = SEL_W                 # strt blocks offset in cs
ST_OFFS = (0, 6, 10, 14)       # per-quarter 2*npq strt col offsets
ONES0 = SEL_W + 16             # all-ones block [6, 3]
CSW = ONES0 + 3
NPAIR = NCH // 2  # 8 chunk pairs
NCORES = 8
NWARM = 33

# wall (packed weights+consts) column offsets
WK0 = 0            # 4 * 256 k-weights, kc-major
WQ0 = KC * FH      # 1024: q weights [128, 64]
V0 = WQ0 + QW      # 1088: v [128, 64] cols (c, i)
TR0 = V0 + 64      # 1152: trit [128, 128]
OH0 = TR0 + CH     # 1280: oneh selectors [128, 72]
OC0 = OH0 + OHW    # 1352: all-ones [128, 128] (tail-quarter T-broadcast)
WALLC = OC0 + CH   # 1480

_BF16 = ml_dtypes.bfloat16

_cache = {}


def _softmax(x, axis=-1):
    e = np.exp(x - x.max(axis=axis, keepdims=True))
    return e / e.sum(axis=axis, keepdims=True)


def _build_program():
    nc = bacc.Bacc()
    f32 = mybir.dt.float32
    bf16 = mybir.dt.bfloat16
    mult = mybir.AluOpType.mult
    add = mybir.AluOpType.add

    wall1_d = nc.dram_tensor("wall1", [CH, V0], bf16, kind="ExternalInput")
    wall2_d = nc.dram_tensor("wall2", [CH, WALLC - V0], bf16, kind="ExternalInput")
    xct_d = nc.dram_tensor("xct", [8, CH, KC, P // 8], bf16, kind="ExternalInput")
    # cs: [sel2 (2x256 in 4 rows) | strtL | strtR | ones]
    cs_d = nc.dram_tensor("cs", [6, CSW], bf16, kind="ExternalInput")
    z_d = nc.dram_tensor("z", [CH, NCH * HPC], f32, kind="ExternalOutput")

    with tile.TileContext(nc) as tc:
        with (
            tc.tile_pool(name="persist", bufs=1) as pers,
            tc.tile_pool(name="work", bufs=3) as work,
            tc.tile_pool(name="pref", bufs=2) as pref,
            tc.tile_pool(name="mixk", bufs=3, space="PSUM") as mixk,
            tc.tile_pool(name="mixq", bufs=2, space="PSUM") as mixq,
            tc.tile_pool(name="smallp", bufs=1, space="PSUM") as smallp,
            tc.tile_pool(name="sp", bufs=2, space="PSUM") as sp,
        ):
            wall_sb = pers.tile([CH, WALLC], bf16, tag="wall_sb")
            cs_sb = pers.tile([6, CSW], bf16, tag="cs_sb")
            xcT = pers.tile([CH, 8, KC, P // 8], bf16, tag="xcT")
            vk_sb = pers.tile([CH, NCH, FH], bf16, tag="vk_sb")
            q_sb = pers.tile([CH, NPAIR, FH2], bf16, tag="q_sb")
            z_sb = pers.tile([CH, NCH * HPC], f32, tag="z_sb")
            dumw = pers.tile([CH, CH], bf16, tag="dumw")

            trit = wall_sb[:, TR0 : TR0 + CH]
            wq_w = wall_sb[:, WQ0 : WQ0 + QW]
            ones11 = cs_sb[0:1, ONES0 : ONES0 + 1]

            # ---- PE warm-up: release the HAM clock gate during the DMA head
            nc.gpsimd.memset(dumw[:], 0.0)
            warmps = smallp.tile([6, FH2], f32, tag="small_ps")
            for i in range(NWARM):
                nc.tensor.matmul(
                    warmps[0:4, 0:CH], dumw[:, 0:4], dumw[:], start=True, stop=True
                )

            # ---- loads: scalar ring carries the one packed weight DMA;
            # sync ring streams x pair-slices (cs squeezed in after xct3).
            nc.sync.dma_start(out=xcT[:, 0], in_=xct_d[0])
            nc.scalar.dma_start(out=wall_sb[:, 0:V0], in_=wall1_d[:])
            nc.scalar.dma_start(out=wall_sb[:, V0:WALLC], in_=wall2_d[:])
            for s in range(1, 8):
                nc.sync.dma_start(out=xcT[:, s], in_=xct_d[s])
                if s == 3:
                    nc.sync.dma_start(out=cs_sb[:], in_=cs_d[:])

            carry_prev = None
            pending_pass2 = None

            def make_pass2(Q):
                npq = NPQS[Q]
                texw_q = texw_tiles.get(Q)
                carry_tail = carry_prev

                def emit():
                    for j in range(npq):
                        Jg = PB[Q] + j
                        psum_S = sp.tile([CH, FH2], f32, tag="psum_S")
                        nc.tensor.matmul(
                            psum_S[:],
                            trit,
                            vk_sb[:, 2 * Jg : 2 * Jg + 2, :].rearrange(
                                "p c f -> p (c f)"
                            ),
                            start=True,
                            stop=False,
                        )
                        if Q == NQ - 1:
                            ones_row = wall_sb[0:1, OC0 : OC0 + CH]
                            nc.tensor.matmul(
                                psum_S[:, 0:FH],
                                ones_row,
                                carry_tail[:],
                                start=False,
                                stop=False,
                            )
                            nc.tensor.matmul(
                                psum_S[:, FH:FH2],
                                ones_row,
                                carry_tail[:],
                                start=False,
                                stop=False,
                            )
                            nc.tensor.matmul(
                                psum_S[:, FH:FH2],
                                wall_sb[:, OC0 : OC0 + CH],
                                vk_sb[:, 2 * Jg, :],
                                start=False,
                                stop=True,
                            )
                        else:
                            nc.tensor.matmul(
                                psum_S[:],
                                cs_sb[0:npq, j * CH : (j + 1) * CH],
                                texw_q[0:npq, :],
                                start=False,
                                stop=True,
                            )
                        prod = work.tile([CH, FH2], bf16, tag="prod")
                        if Jg >= NPAIR - 2:
                            # tail pair: multiply straight from PSUM (skip
                            # the s_sb copy; latency beats throughput here)
                            nc.vector.tensor_tensor(
                                out=prod[:], in0=q_sb[:, Jg, :], in1=psum_S[:],
                                op=mult,
                            )
                        else:
                            s_sb = work.tile([CH, FH2], bf16, tag="s_sb")
                            nc.scalar.copy(s_sb[:], psum_S[:])
                            nc.vector.tensor_tensor(
                                out=prod[:], in0=q_sb[:, Jg, :], in1=s_sb[:],
                                op=mult,
                            )
                        # cols are (c, i, t, g): reduce contiguous (t,g)=64
                        nc.vector.tensor_reduce(
                            out=z_sb[:, 2 * Jg * HPC : (2 * Jg + 2) * HPC],
                            in_=prod[:].rearrange("p (a x) -> p a x", x=NT * GL),
                            axis=mybir.AxisListType.X,
                            op=add,
                        )
                    zo = CB[Q] * HPC
                    zw = CPQS[Q] * HPC
                    nc.sync.dma_start(
                        out=z_d[:, zo : zo + zw], in_=z_sb[:, zo : zo + zw]
                    )

                return emit

            texw_tiles = {}
            for Q in range(NQ):
                cpq, npq = CPQS[Q], NPQS[Q]
                lastq = Q == NQ - 1
                t2ps = None if lastq else smallp.tile([6, FH2], f32, tag="small_ps")
                for j in range(npq):  # pairs within quarter
                    Jg = PB[Q] + j
                    pkk = mixk.tile([CH, FH2], f32, tag="pkk")
                    pqq = mixq.tile([CH, FH2], f32, tag="pqq")
                    pqv = pqq[:].rearrange(
                        "p (c i t g) -> p c i t g", c=2, i=HPC, t=NT, g=GL
                    )
                    for cl2 in range(2):
                        c = 2 * Jg + cl2
                        s, off = c // 2, (c % 2) * CH
                        first = cl2 == 0
                        last = cl2 == 1
                        for kc in range(KC):
                            xst = xcT[:, s, kc, off : off + CH]
                            nc.tensor.matmul(
                                pkk[:, cl2 * FH : (cl2 + 1) * FH],
                                xst,
                                wall_sb[:, kc * FH : (kc + 1) * FH],
                                start=(first and kc == 0),
                                stop=(last and kc == KC - 1),
                                skip_group_check=True,
                            )
                            nc.tensor.matmul(
                                pqv[:, cl2, :, kc, :],
                                xst,
                                wq_w,
                                start=(first and kc == 0),
                                stop=(last and kc == KC - 1),
                                skip_group_check=True,
                            )
                    # vk = ktilde * v  (one TT per pair; v bcast over (t,g))
                    nc.vector.tensor_tensor(
                        out=vk_sb[:, 2 * Jg : 2 * Jg + 2, :].rearrange(
                            "p c (i x) -> p c i x", i=HPC
                        ),
                        in0=pkk[:].rearrange("p (c i x) -> p c i x", c=2, i=HPC),
                        in1=wall_sb[:, V0 + 2 * Jg * HPC : V0 + (2 * Jg + 2) * HPC]
                        .rearrange("p (c i) -> p c i", c=2)
                        .unsqueeze(3)
                        .broadcast_to([CH, 2, HPC, NT * GL]),
                        op=mult,
                    )
                    # split the q drain across ACT and DVE: frees the q
                    # psum bank in half the time and balances both queues
                    nc.scalar.copy(q_sb[:, Jg, 0:FH], pqq[:, 0:FH])
                    nc.vector.tensor_copy(q_sb[:, Jg, FH:FH2], pqq[:, FH:FH2])
                    # per-chunk column sums (not needed for the tail quarter)
                    for cl2 in range(2) if not lastq else ():
                        cl = 2 * j + cl2
                        nc.tensor.matmul(
                            t2ps[0:cpq, 0:FH],
                            wall_sb[
                                :, OH0 + OH_OFF[Q] + cl * cpq : OH0
                                + OH_OFF[Q] + (cl + 1) * cpq
                            ],
                            vk_sb[:, CB[Q] + cl, :],
                            start=(cl == 0),
                            stop=(cl == cpq - 1),
                        )

                # ---- prefix for this quarter (chunk-granular cumsums).
                # The tail quarter (1 pair) needs none of it: its TexL IS
                # carry_prev and TexR arrives as a PE broadcast-matmul.
                if lastq:
                    if pending_pass2 is not None:
                        pending_pass2()
                    pending_pass2 = make_pass2(Q)
                    continue
                t2q_sb = pref.tile([6, FH], bf16, tag="t2q_sb")
                nc.scalar.copy(t2q_sb[0:cpq, :], t2ps[0:cpq, 0:FH])
                tps = smallp.tile([6, FH2], f32, tag="small_ps")
                soff = ST_OFF + ST_OFFS[Q]
                strtLq = cs_sb[0:cpq, soff : soff + npq]
                strtRq = cs_sb[0:cpq, soff + npq : soff + 2 * npq]
                ones_1n = cs_sb[0:1, ONES0 : ONES0 + npq]
                ones_n1 = cs_sb[0:cpq, ONES0 : ONES0 + 1]
                last = carry_prev is None
                nc.tensor.matmul(
                    tps[0:npq, 0:FH], strtLq, t2q_sb[0:cpq, :], start=True, stop=last
                )
                if carry_prev is not None:
                    nc.tensor.matmul(
                        tps[0:npq, 0:FH], ones_1n, carry_prev[:], start=False,
                        stop=True,
                    )
                nc.tensor.matmul(
                    tps[0:npq, FH:FH2], strtRq, t2q_sb[0:cpq, :], start=True,
                    stop=last,
                )
                if carry_prev is not None:
                    nc.tensor.matmul(
                        tps[0:npq, FH:FH2], ones_1n, carry_prev[:], start=False,
                        stop=True,
                    )
                texw_sb = pref.tile([3, FH2], bf16, tag="texw_sb")
                nc.scalar.copy(texw_sb[0:npq, :], tps[0:npq, :])
                texw_tiles[Q] = texw_sb
                if Q < NQ - 1:
                    # carry accumulates in the t2 bank (free region)
                    nc.tensor.matmul(
                        t2ps[0:1, FH:FH2], ones_n1, t2q_sb[0:cpq, :], start=True,
                        stop=last,
                    )
                    if carry_prev is not None:
                        nc.tensor.matmul(
                            t2ps[0:1, FH:FH2],
                            ones11,
                            carry_prev[:],
                            start=False,
                            stop=True,
                        )
                    carry_new = pref.tile([1, FH], bf16, tag="carry_sb")
                    nc.scalar.copy(carry_new[:], t2ps[0:1, FH:FH2])
                    carry_prev = carry_new

                # emit the PREVIOUS quarter's pass 2 now: its prefix chain has
                # settled, and the current quarter's chain ops keep priority
                if pending_pass2 is not None:
                    pending_pass2()
                pending_pass2 = make_pass2(Q)

            pending_pass2()  # last quarter's pass 2


    nc.finalize()
    return nc


def _host_inputs(x_cat, x_num, W_K, W_Q, W_pred, W_V):
    """Per-core input maps. Core c = batch (c//2), head-group (c%2)."""
    pk = _softmax(W_K.astype(np.float64)).astype(np.float32)
    pq = _softmax(W_Q.astype(np.float64)).astype(np.float32)
    pp = _softmax(W_pred.astype(np.float64)).astype(np.float32)
    pv = _softmax(W_V.astype(np.float64)).astype(np.float32)

    v_full = np.einsum("bpd,id->bpi", x_num, pv)  # [B, P, H] fp32

    # constants (cs [6, CSW]): sel rows (up to 3 pairs), per-quarter strt
    # blocks, all-ones block
    cs = np.zeros((6, CSW), np.float32)
    for j in range(3):
        cs[j, j * CH : (j + 1) * CH] = 1.0
    for Q in range(NQ):
        cpq, npq = CPQS[Q], NPQS[Q]
        soff = ST_OFF + ST_OFFS[Q]
        for k in range(cpq):
            for m in range(npq):
                cs[k, soff + m] = 1.0 if k < 2 * m else 0.0
                cs[k, soff + npq + m] = 1.0 if k <= 2 * m else 0.0
    cs[:, ONES0 : ONES0 + 3] = 1.0

    trit = np.triu(np.ones((CH, CH), np.float32))
    oneh = np.zeros((CH, OHW), np.float32)
    for Q in range(NQ):
        cpq = CPQS[Q]
        for cl in range(cpq):
            oneh[:, OH_OFF[Q] + cl * cpq + cl] = 1.0

    hh = np.arange(DC) // NV
    vv = np.arange(DC) % NV

    in_maps = []
    for core in range(NCORES):
        b, hg = core // 2, core % 2
        heads = [hg * HPC + j for j in range(HPC)]
        # h-major features: f' = h*NV + v
        x_hm = x_cat[b].reshape(P, NV, DH).transpose(0, 2, 1).reshape(P, DC)
        xct = np.ascontiguousarray(
            x_hm.T.reshape(KC, CH, 8, P // 8).transpose(2, 1, 0, 3)
        ).astype(_BF16)

        # Wk [512 (h,v), (i, t, glo)]: pk[i,v] * pp[i, t*16+glo, h]
        Wk = np.zeros((DC, HPC, NT, GL), np.float32)
        for il, hd in enumerate(heads):
            ppT = pp[hd].T  # [g, h_out]
            Wk[:, il, :, :] = pk[hd][vv][:, None, None] * ppT[hh].reshape(DC, NT, GL)
        wk = Wk.reshape(DC, FH).reshape(KC, CH, FH).transpose(1, 0, 2).reshape(
            CH, KC * FH
        )

        # Wq [128 (hl, v), (i, hlo)]
        wq = np.einsum(
            "hg,iv->hvig", np.eye(GL, dtype=np.float32), pq[heads]
        ).reshape(CH, QW)

        # v [128, (c, i)]
        v_core = v_full[b][:, heads]  # [P, 4]
        v_dev = v_core.reshape(NCH, CH, HPC).transpose(1, 0, 2).reshape(CH, NCH * HPC)

        wall1 = np.concatenate([wk, wq], axis=1)          # [128, 1088]
        wall2 = np.concatenate(
            [v_dev, trit, oneh, np.ones((CH, CH), np.float32)], axis=1
        )  # [128, 392]

        in_maps.append(
            {
                "xct": xct,
                "wall1": np.ascontiguousarray(wall1).astype(_BF16),
                "wall2": np.ascontiguousarray(wall2).astype(_BF16),
                "cs": cs.astype(_BF16),
            }
        )
    return in_maps


def _run(inputs, **spmd_kwargs):
    if "nc" not in _cache:
        _cache["nc"] = _build_program()
    nc = _cache["nc"]

    in_maps = _host_inputs(**inputs)
    res = run_bass_kernel_spmd(nc, in_maps, list(range(NCORES)), **spmd_kwargs)

    out = np.zeros((B, P, H), np.float32)
    for core in range(NCORES):
        b, hg = core // 2, core % 2
        z = res.results[core]["z"]  # [128, NCH*HPC]
        z = z.reshape(CH, NCH, HPC).transpose(1, 0, 2).reshape(P, HPC)
        out[b, :, hg * HPC : (hg + 1) * HPC] = z
    return out, res


def kernel(x_cat, x_num, W_K, W_Q, W_pred, W_V):
    out, _ = _run(
        dict(x_cat=x_cat, x_num=x_num, W_K=W_K, W_Q=W_Q, W_pred=W_pred, W_V=W_V)
    )
    return out


# revision 22
# speedup vs baseline: 1.0300x; 1.0300x over previous
"""Trainium2 Bass kernel for nn_NumAttention (sparse_attention).

Reference computation (per batch b, head i):
    k     = blockmix(x_cat, softmax(W_K)[i])            # [P, DH]
    xq    = blockmix(x_cat, softmax(W_Q)[i])            # [P, DH]
    q     = xq @ softmax(W_pred)[i]                     # [P, DH]
    v     = x_num @ softmax(W_V)[i]                     # [P]
    z[qp] = sum_{p<=qp} v[p] * (k[p] . q[qp])           # causal, no softmax

The attention is softmax-free with scalar values, hence linear:
z[qp] = xq[qp] . S[qp] with S = cumsum_p(v[p] * ktilde[p,:]),
ktilde = k @ pp^T folded into one mix weight.  O(P^2) scores never exist.

Design notes:
  * Features are reordered h-major (f' = h*NV + v) so the q-side weight
    pq (x) I is block-diagonal: each 128-feature K-tile feeds exactly 64
    q columns -> four N=64 matmuls per chunk instead of a dense N=256
    stream.  Mix = 20480 PE columns instead of 32768.
  * ktilde/q columns are ordered (chunk, head, t, glo): the final
    z[p,(c,i)] = sum_64(q * S) is one tensor_tensor + one tensor_reduce
    per chunk-pair, writing z columns directly (no fold pass).  The q
    matmuls scatter into a strided PSUM view to interleave t=kc.
  * 16 chunks run as four uneven quarters (6/4/4/2) with a sequential
    inter-quarter carry: pass-2 of quarter Q overlaps the mix of Q+1, and
    the tiny tail quarter skips the whole prefix pipeline (its TexL IS the
    carry; the odd chunk's total arrives as a ones-matrix broadcast matmul
    straight into PSUM).
  * Mix psum is allocated per chunk-PAIR (one k bank, one q bank) so the
    post-matmul DVE/ACT traffic is one vk-multiply + one q-copy per pair.
  * All weights+constants ship as ONE [128, 1296] DMA (2.6KB/partition
    descriptors); x ships as 8 pair-slices on the other HWDGE ring.

Sharding: 8 cores = 4 batches x 2 head-groups (4 heads each).  Host ships
x_cat[b] transposed h-major bf16 and host-computed v (x_num @ pv, 8 MFLOP).
A burst of skinny dummy matmuls warms the PE HAM clock gate during the DMA
head.
"""

import numpy as np
import ml_dtypes

import concourse.bacc as bacc
import concourse.mybir as mybir
import concourse.tile as tile
from concourse.bass_utils import run_bass_kernel_spmd

B, P, DC, DN, H, DH = 4, 2048, 512, 64, 8, 64
NV = DC // DH     # 8 variables
CH = 128          # positions per chunk
NCH = P // CH     # 16 chunks
HPC = 4           # heads per core
NT = 4            # h-groups of 16 (= KC)
GL = 16           # h's per group
FH = HPC * DH     # 256 = ktilde/q width per core, cols ordered (i, t, glo)
FH2 = 2 * FH      # 512 = chunk-pair width
KC = DC // CH     # 4 feature K-tiles
QW = HPC * GL     # 64 q-columns per K-tile
NQ = 4            # quarters (uneven: big early, tiny last for a short tail)
CPQS = (6, 4, 4, 2)            # chunks per quarter
NPQS = (3, 2, 2, 1)            # pairs per quarter
CB = (0, 6, 10, 14)            # chunk base per quarter
PB = (0, 3, 5, 7)              # pair base per quarter
OH_OFF = (0, 36, 52, 68)       # oneh col offset per quarter (cpq^2 widths)
OHW = 72
SEL_W = 3 * CH                 # sel block in cs (up to 3 pairs)
ST_OFF = SEL_W                 # strt blocks offset in cs
ST_OFFS = (0, 6, 10, 14)       # per-quarter 2*npq strt col offsets
ONES0 = SEL_W + 16             # all-ones block [6, 3]
CSW = ONES0 + 3
NPAIR = NCH // 2  # 8 chunk pairs
NCORES = 8
NWARM = 33

# wall (packed weights+consts) column offsets
WK0 = 0            # 4 * 256 k-weights, kc-major
WQ0 = KC * FH      # 1024: q weights [128, 64]
V0 = WQ0 + QW      # 1088: v [128, 64] cols (c, i)
TR0 = V0 + 64      # 1152: trit [128, 128]
OH0 = TR0 + CH     # 1280: oneh selectors [128, 72]
OC0 = OH0 + OHW    # 1352: all-ones [128, 128] (tail-quarter T-broadcast)
WALLC = OC0 + CH   # 1480

_BF16 = ml_dtypes.bfloat16

_cache = {}


def _softmax(x, axis=-1):
    e = np.exp(x - x.max(axis=axis, keepdims=True))
    return e / e.sum(axis=axis, keepdims=True)


def _build_program():
    nc = bacc.Bacc()
    f32 = mybir.dt.float32
    bf16 = mybir.dt.bfloat16
    mult = mybir.AluOpType.mult
    add = mybir.AluOpType.add

    wall1_d = nc.dram_tensor("wall1", [CH, V0], bf16, kind="ExternalInput")
    wall2_d = nc.dram_tensor("wall2", [CH, WALLC - V0], bf16, kind="ExternalInput")
    xct_d = nc.dram_tensor("xct", [8, CH, KC, P // 8], bf16, kind="ExternalInput")
    # cs: [sel2 (2x256 in 4 rows) | strtL | strtR | ones]
    cs_d = nc.dram_tensor("cs", [6, CSW], bf16, kind="ExternalInput")
    z_d = nc.dram_tensor("z", [CH, NCH * HPC], f32, kind="ExternalOutput")

    with tile.TileContext(nc) as tc:
        with (
            tc.tile_pool(name="persist", bufs=1) as pers,
            tc.tile_pool(name="work", bufs=3) as work,
            tc.tile_pool(name="pref", bufs=2) as pref,
            tc.tile_pool(name="mixk", bufs=2, space="PSUM") as mixk,
            tc.tile_pool(name="mixq", bufs=2, space="PSUM") as mixq,
            tc.tile_pool(name="t2p", bufs=1, space="PSUM") as t2p,
            tc.tile_pool(name="psmall", bufs=1, space="PSUM") as psmall,
            tc.tile_pool(name="sp", bufs=2, space="PSUM") as sp,
        ):
            wall_sb = pers.tile([CH, WALLC], bf16, tag="wall_sb")
            cs_sb = pers.tile([6, CSW], bf16, tag="cs_sb")
            xcT = pers.tile([CH, 8, KC, P // 8], bf16, tag="xcT")
            vk_sb = pers.tile([CH, NCH, FH], bf16, tag="vk_sb")
            q_sb = pers.tile([CH, NPAIR, FH2], bf16, tag="q_sb")
            z_sb = pers.tile([CH, NCH * HPC], f32, tag="z_sb")
            dumw = pers.tile([CH, CH], bf16, tag="dumw")

            trit = wall_sb[:, TR0 : TR0 + CH]
            wq_w = wall_sb[:, WQ0 : WQ0 + QW]
            ones11 = cs_sb[0:1, ONES0 : ONES0 + 1]

            # ---- PE warm-up: release the HAM clock gate during the DMA head
            nc.gpsimd.memset(dumw[:], 0.0)
            warmps = psmall.tile([4, FH2], f32, tag="texw_ps")
            for i in range(NWARM):
                nc.tensor.matmul(
                    warmps[:, 0:CH], dumw[:, 0:4], dumw[:], start=True, stop=True
                )

            # ---- loads: scalar ring carries the one packed weight DMA;
            # sync ring streams x pair-slices (cs squeezed in after xct3).
            nc.sync.dma_start(out=xcT[:, 0], in_=xct_d[0])
            nc.scalar.dma_start(out=wall_sb[:, 0:V0], in_=wall1_d[:])
            nc.scalar.dma_start(out=wall_sb[:, V0:WALLC], in_=wall2_d[:])
            for s in range(1, 8):
                nc.sync.dma_start(out=xcT[:, s], in_=xct_d[s])
                if s == 3:
                    nc.sync.dma_start(out=cs_sb[:], in_=cs_d[:])

            carry_prev = None
            pending_pass2 = None

            def make_pass2(Q):
                npq = NPQS[Q]
                texw_q = texw_tiles.get(Q)
                carry_tail = carry_prev

                def emit():
                    for j in range(npq):
                        Jg = PB[Q] + j
                        psum_S = sp.tile([CH, FH2], f32, tag="psum_S")
                        nc.tensor.matmul(
                            psum_S[:],
                            trit,
                            vk_sb[:, 2 * Jg : 2 * Jg + 2, :].rearrange(
                                "p c f -> p (c f)"
                            ),
                            start=True,
                            stop=False,
                        )
                        if Q == NQ - 1:
                            ones_row = wall_sb[0:1, OC0 : OC0 + CH]
                            nc.tensor.matmul(
                                psum_S[:, 0:FH],
                                ones_row,
                                carry_tail[:],
                                start=False,
                                stop=False,
                            )
                            nc.tensor.matmul(
                                psum_S[:, FH:FH2],
                                ones_row,
                                carry_tail[:],
                                start=False,
                                stop=False,
                            )
                            nc.tensor.matmul(
                                psum_S[:, FH:FH2],
                                wall_sb[:, OC0 : OC0 + CH],
                                vk_sb[:, 2 * Jg, :],
                                start=False,
                                stop=True,
                            )
                        else:
                            nc.tensor.matmul(
                                psum_S[:],
                                cs_sb[0:npq, j * CH : (j + 1) * CH],
                                texw_q[0:npq, :],
                                start=False,
                                stop=True,
                            )
                        prod = work.tile([CH, FH2], bf16, tag="prod")
                        if Jg >= NPAIR - 2:
                            # tail pair: multiply straight from PSUM (skip
                            # the s_sb copy; latency beats throughput here)
                            nc.vector.tensor_tensor(
                                out=prod[:], in0=q_sb[:, Jg, :], in1=psum_S[:],
                                op=mult,
                            )
                        else:
                            s_sb = work.tile([CH, FH2], bf16, tag="s_sb")
                            nc.scalar.copy(s_sb[:], psum_S[:])
                            nc.vector.tensor_tensor(
                                out=prod[:], in0=q_sb[:, Jg, :], in1=s_sb[:],
                                op=mult,
                            )
                        # cols are (c, i, t, g): reduce contiguous (t,g)=64
                        nc.vector.tensor_reduce(
                            out=z_sb[:, 2 * Jg * HPC : (2 * Jg + 2) * HPC],
                            in_=prod[:].rearrange("p (a x) -> p a x", x=NT * GL),
                            axis=mybir.AxisListType.X,
                            op=add,
                        )
                    zo = CB[Q] * HPC
                    zw = CPQS[Q] * HPC
                    nc.sync.dma_start(
                        out=z_d[:, zo : zo + zw], in_=z_sb[:, zo : zo + zw]
                    )

                return emit

            texw_tiles = {}
            for Q in range(NQ):
                cpq, npq = CPQS[Q], NPQS[Q]
                lastq = Q == NQ - 1
                t2ps = None if lastq else t2p.tile([6, FH2], f32, tag="t2ps")
                for j in range(npq):  # pairs within quarter
                    Jg = PB[Q] + j
                    pkk = mixk.tile([CH, FH2], f32, tag="pkk")
                    pqq = mixq.tile([CH, FH2], f32, tag="pqq")
                    pqv = pqq[:].rearrange(
                        "p (c i t g) -> p c i t g", c=2, i=HPC, t=NT, g=GL
                    )
                    for cl2 in range(2):
                        c = 2 * Jg + cl2
                        s, off = c // 2, (c % 2) * CH
                        first = cl2 == 0
                        last = cl2 == 1
                        for kc in range(KC):
                            xst = xcT[:, s, kc, off : off + CH]
                            nc.tensor.matmul(
                                pkk[:, cl2 * FH : (cl2 + 1) * FH],
                                xst,
                                wall_sb[:, kc * FH : (kc + 1) * FH],
                                start=(first and kc == 0),
                                stop=(last and kc == KC - 1),
                                skip_group_check=True,
                            )
                            nc.tensor.matmul(
                                pqv[:, cl2, :, kc, :],
                                xst,
                                wq_w,
                                start=(first and kc == 0),
                                stop=(last and kc == KC - 1),
                                skip_group_check=True,
                            )
                    # vk = ktilde * v  (one TT per pair; v bcast over (t,g))
                    nc.vector.tensor_tensor(
                        out=vk_sb[:, 2 * Jg : 2 * Jg + 2, :].rearrange(
                            "p c (i x) -> p c i x", i=HPC
                        ),
                        in0=pkk[:].rearrange("p (c i x) -> p c i x", c=2, i=HPC),
                        in1=wall_sb[:, V0 + 2 * Jg * HPC : V0 + (2 * Jg + 2) * HPC]
                        .rearrange("p (c i) -> p c i", c=2)
                        .unsqueeze(3)
                        .broadcast_to([CH, 2, HPC, NT * GL]),
                        op=mult,
                    )
                    # split the q drain across ACT and DVE: frees the q
                    # psum bank in half the time and balances both queues
                    nc.scalar.copy(q_sb[:, Jg, 0:FH], pqq[:, 0:FH])
                    nc.vector.tensor_copy(q_sb[:, Jg, FH:FH2], pqq[:, FH:FH2])
                    # per-chunk column sums (not needed for the tail quarter)
                    for cl2 in range(2) if not lastq else ():
                        cl = 2 * j + cl2
                        nc.tensor.matmul(
                            t2ps[0:cpq, 0:FH],
                            wall_sb[
                                :, OH0 + OH_OFF[Q] + cl * cpq : OH0
                                + OH_OFF[Q] + (cl + 1) * cpq
                            ],
                            vk_sb[:, CB[Q] + cl, :],
                            start=(cl == 0),
                            stop=(cl == cpq - 1),
                        )

                # ---- prefix for this quarter (chunk-granular cumsums).
                # The tail quarter (1 pair) needs none of it: its TexL IS
                # carry_prev and TexR arrives as a PE broadcast-matmul.
                if lastq:
                    if pending_pass2 is not None:
                        pending_pass2()
                    pending_pass2 = make_pass2(Q)
                    continue
                t2q_sb = pref.tile([6, FH], bf16, tag="t2q_sb")
                nc.scalar.copy(t2q_sb[0:cpq, :], t2ps[0:cpq, 0:FH])
                tps = psmall.tile([6, FH2], f32, tag="texw_ps")
                soff = ST_OFF + ST_OFFS[Q]
                strtLq = cs_sb[0:cpq, soff : soff + npq]
                strtRq = cs_sb[0:cpq, soff + npq : soff + 2 * npq]
                ones_1n = cs_sb[0:1, ONES0 : ONES0 + npq]
                ones_n1 = cs_sb[0:cpq, ONES0 : ONES0 + 1]
                last = carry_prev is None
                nc.tensor.matmul(
                    tps[0:npq, 0:FH], strtLq, t2q_sb[0:cpq, :], start=True, stop=last
                )
                if carry_prev is not None:
                    nc.tensor.matmul(
                        tps[0:npq, 0:FH], ones_1n, carry_prev[:], start=False,
                        stop=True,
                    )
                nc.tensor.matmul(
                    tps[0:npq, FH:FH2], strtRq, t2q_sb[0:cpq, :], start=True,
                    stop=last,
                )
                if carry_prev is not None:
                    nc.tensor.matmul(
                        tps[0:npq, FH:FH2], ones_1n, carry_prev[:], start=False,
                        stop=True,
                    )
                texw_sb = pref.tile([3, FH2], bf16, tag="texw_sb")
                nc.scalar.copy(texw_sb[0:npq, :], tps[0:npq, :])
                texw_tiles[Q] = texw_sb
                if Q < NQ - 1:
                    # carry accumulates in the t2 bank (free region)
                    nc.tensor.matmul(
                        t2ps[0:1, FH:FH2], ones_n1, t2q_sb[0:cpq, :], start=True,
                        stop=last,
                    )
                    if carry_prev is not None:
                        nc.tensor.matmul(
                            t2ps[0:1, FH:FH2],
                            ones11,
                            carry_prev[:],
                            start=False,
                            stop=True,
                        )
                    carry_new = pref.tile([1, FH], bf16, tag="carry_sb")
                    nc.scalar.copy(carry_new[:], t2ps[0:1, FH:FH2])
                    carry_prev = carry_new

                # emit the PREVIOUS quarter's pass 2 now: its prefix chain has
                # settled, and the current quarter's chain ops keep priority
                if pending_pass2 is not None:
                    pending_pass2()
                pending_pass2 = make_pass2(Q)

            pending_pass2()  # last quarter's pass 2


    nc.finalize()
    return nc


def _host_inputs(x_cat, x_num, W_K, W_Q, W_pred, W_V):
    """Per-core input maps. Core c = batch (c//2), head-group (c%2)."""
    pk = _softmax(W_K.astype(np.float64)).astype(np.float32)
    pq = _softmax(W_Q.astype(np.float64)).astype(np.float32)
    pp = _softmax(W_pred.astype(np.float64)).astype(np.float32)
    pv = _softmax(W_V.astype(np.float64)).astype(np.float32)

    v_full = np.einsum("bpd,id->bpi", x_num, pv)  # [B, P, H] fp32

    # constants (cs [6, CSW]): sel rows (up to 3 pairs), per-quarter strt
    # blocks, all-ones block
    cs = np.zeros((6, CSW), np.float32)
    for j in range(3):
        cs[j, j * CH : (j + 1) * CH] = 1.0
    for Q in range(NQ):
        cpq, npq = CPQS[Q], NPQS[Q]
        soff = ST_OFF + ST_OFFS[Q]
        for k in range(cpq):
            for m in range(npq):
                cs[k, soff + m] = 1.0 if k < 2 * m else 0.0
                cs[k, soff + npq + m] = 1.0 if k <= 2 * m else 0.0
    cs[:, ONES0 : ONES0 + 3] = 1.0

    trit = np.triu(np.ones((CH, CH), np.float32))
    oneh = np.zeros((CH, OHW), np.float32)
    for Q in range(NQ):
        cpq = CPQS[Q]
        for cl in range(cpq):
            oneh[:, OH_OFF[Q] + cl * cpq + cl] = 1.0

    hh = np.arange(DC) // NV
    vv = np.arange(DC) % NV

    in_maps = []
    for core in range(NCORES):
        b, hg = core // 2, core % 2
        heads = [hg * HPC + j for j in range(HPC)]
        # h-major features: f' = h*NV + v
        x_hm = x_cat[b].reshape(P, NV, DH).transpose(0, 2, 1).reshape(P, DC)
        xct = np.ascontiguousarray(
            x_hm.T.reshape(KC, CH, 8, P // 8).transpose(2, 1, 0, 3)
        ).astype(_BF16)

        # Wk [512 (h,v), (i, t, glo)]: pk[i,v] * pp[i, t*16+glo, h]
        Wk = np.zeros((DC, HPC, NT, GL), np.float32)
        for il, hd in enumerate(heads):
            ppT = pp[hd].T  # [g, h_out]
            Wk[:, il, :, :] = pk[hd][vv][:, None, None] * ppT[hh].reshape(DC, NT, GL)
        wk = Wk.reshape(DC, FH).reshape(KC, CH, FH).transpose(1, 0, 2).reshape(
            CH, KC * FH
        )

        # Wq [128 (hl, v), (i, hlo)]
        wq = np.einsum(
            "hg,iv->hvig", np.eye(GL, dtype=np.float32), pq[heads]
        ).reshape(CH, QW)

        # v [128, (c, i)]
        v_core = v_full[b][:, heads]  # [P, 4]
        v_dev = v_core.reshape(NCH, CH, HPC).transpose(1, 0, 2).reshape(CH, NCH * HPC)

        wall1 = np.concatenate([wk, wq], axis=1)          # [128, 1088]
        wall2 = np.concatenate(
            [v_dev, trit, oneh, np.ones((CH, CH), np.float32)], axis=1
        )  # [128, 392]

        in_maps.append(
            {
                "xct": xct,
                "wall1": np.ascontiguousarray(wall1).astype(_BF16),
                "wall2": np.ascontiguousarray(wall2).astype(_BF16),
                "cs": cs.astype(_BF16),
            }
        )
    return in_maps


def _run(inputs, **spmd_kwargs):
    if "nc" not in _cache:
        _cache["nc"] = _build_program()
    nc = _cache["nc"]

    in_maps = _host_inputs(**inputs)
    res = run_bass_kernel_spmd(nc, in_maps, list(range(NCORES)), **spmd_kwargs)

    out = np.zeros((B, P, H), np.float32)
    for core in range(NCORES):
        b, hg = core // 2, core % 2
        z = res.results[core]["z"]  # [128, NCH*HPC]
        z = z.reshape(CH, NCH, HPC).transpose(1, 0, 2).reshape(P, HPC)
        out[b, :, hg * HPC : (hg + 1) * HPC] = z
    return out, res


def kernel(x_cat, x_num, W_K, W_Q, W_pred, W_V):
    out, _ = _run(
        dict(x_cat=x_cat, x_num=x_num, W_K=W_K, W_Q=W_Q, W_pred=W_pred, W_V=W_V)
    )
    return out
